# revision 15
# baseline (speedup 1.0000x reference)
import sys
sys.path.insert(0, "/opt/trn_rl_repo")
"""CapsuleBlock kernel for TRN2, i-sharded across 8 cores (v3: resident u_hat).

Per-core (NI=512 input capsules local):
  u = squash(x); u_hat[b,c,i,j] computed ONCE via block-diagonal-u matmuls
  (K=(i'4,d)=64, M=(i4,b)=128, N=(c,j)=512) and stored bf16 in SBUF as
  u_hat[(i4,b), (igH, c, j, igL)] with local i = 4*(igH*IGL+igL) + i4.
  s0 (uniform coupling) is accumulated on PE during the same W stream.
  Routing runs on DVE/GpSimd:
    s-pass: prod = cw (.) u_hat (j-major prod), one X-reduce over (igH,igL),
            i4-partition-reduce via mask matmul, 64KB AllReduce, squash.
    b-pass: prod2 = v (.) u_hat, j-tree-add, bl += ., softmax over c -> cw.
  A dummy AllReduce at kernel start absorbs the cold-collective cost.

Host-side prep (layout only): x fed as [(b,iq)=128, (ir,d)=2048] f32;
W fed pre-transposed/cast as [(gpar,i4,d)=128, (g2,c,j)=32768] bf16.
"""

import os
import numpy as np
from contextlib import ExitStack

import concourse.bass as bass
import concourse.mybir as mybir
import concourse.tile as tile
from concourse import masks

f32 = mybir.dt.float32
bf16 = mybir.dt.bfloat16
AX = mybir.AxisListType
OP = mybir.AluOpType
ACTF = mybir.ActivationFunctionType

B, C, J, D = 32, 32, 16, 16
ROUTINGS = 3
EPS = 1e-7
N_CORES = 8
NI = 4096 // N_CORES          # 512 per core
G = NI // 4                   # 128 groups of 4 i's
G2 = G // 2                   # 64
IGH, IGL = 32, 4              # g = igH*IGL + igL
CJ = C * J                    # 512
REPL = [list(range(N_CORES))]


def build_capsule_kernel(tc: tile.TileContext, v_out: bass.AP, x_in: bass.AP,
                         w_in: bass.AP):
    """v_out [B, C, J] f32; x_in [128, 2048] f32; w_in [128, G2*CJ] bf16."""
    nc = tc.nc
    no_cc = os.environ.get("NO_COLLECTIVE") == "1"

    ctx = ExitStack()
    main = ctx.enter_context(tc.tile_pool(name="main", bufs=1))
    psp = ctx.enter_context(tc.tile_pool(name="ps", bufs=2, space="PSUM"))
    evp = ctx.enter_context(tc.tile_pool(name="evp", bufs=5, space="PSUM"))
    s0p = ctx.enter_context(tc.tile_pool(name="s0p", bufs=1, space="PSUM"))
    dram = ctx.enter_context(tc.tile_pool(name="dram", bufs=1, space="DRAM"))

    # ---------- persistent SBUF ----------
    u_hat = main.tile([128, IGH * C * J * IGL], bf16)   # 128KB/p
    bl = main.tile([128, IGH * C * IGL], f32)           # 16KB/p
    cw = main.tile([128, IGH * C * IGL], bf16)          # 8KB/p
    s_pre = main.tile([128, CJ], bf16)                  # 1KB/p
    zsum = main.tile([128, IGH * IGL], f32)
    zi = main.tile([128, IGH * IGL], f32)
    zibf = main.tile([128, IGH * IGL], bf16)
    v_rep = main.tile([128, CJ], bf16)
    s_sb = main.tile([32, CJ], f32)
    v_sb = main.tile([32, CJ], f32)
    v_sq = main.tile([32, CJ], f32)
    vbf = main.tile([32, CJ], bf16)
    sqs = main.tile([32, 4 * C], f32)
    msk = main.tile([128, 32], bf16)
    cst = main.tile([128, 2], f32)

    nc.vector.memset(cst[:, 0:1], EPS)
    nc.vector.memset(cst[:, 1:2], 0.0)
    nc.vector.memset(bl[:], 0.0)

    # warm up the collective ring early; latency hides under init
    if not no_cc:
        war_i = dram.tile([32, 2], f32, tag="wari")
        war_o = dram.tile([32, 2], f32, tag="waro")
        nc.sync.dma_start(war_i[:], cst[0:32, 0:2])
        nc.gpsimd.collective_compute(
            "AllReduce", OP.add, replica_groups=REPL,
            ins=[war_i.opt()], outs=[war_o.opt()])

    # ---------- init: x -> u -> uT -> BD; stream W -> u_hat, s0 ----------
    with tc.tile_pool(name="init2", bufs=1) as initp2:
        ident = initp2.tile([128, 128], f32)
        ident_bf = initp2.tile([32, 32], bf16)
        uT = initp2.tile([128, G2 * B], bf16)    # [(gpar,i4,d), (g2,b)]
        BD = initp2.tile([128, G2 * 128], bf16)  # [(gpar,i4,d), (g2,i4',b)]
        masks.make_identity(nc, ident[:])
        masks.make_identity(nc, ident_bf[:])
        for k4 in range(4):
            nc.sync.dma_start(msk[32 * k4:32 * (k4 + 1), :], ident_bf[:])
        nc.vector.memset(BD[:], 0.0)

        with tc.tile_pool(name="init1", bufs=1) as initp1:
            x_sb = initp1.tile([128, 2048], f32)     # [(b,iq), (ir,d)]
            u_b = initp1.tile([128, 2048], f32)
            sq = initp1.tile([128, 512], f32)
            nc.sync.dma_start(x_sb[:], x_in[:])

            # squash over d for each of 128 local-i per partition
            nc.vector.tensor_mul(u_b[:], x_sb[:], x_sb[:])
            s0 = sq[:, 0:128]
            nc.vector.tensor_reduce(s0,
                                    u_b[:].rearrange("p (i d) -> p i d", d=D),
                                    axis=AX.X, op=OP.add)
            t1 = sq[:, 128:256]
            nc.scalar.activation(t1, s0, ACTF.Sqrt, bias=cst[:, 0:1],
                                 scale=1.0)
            t2 = sq[:, 256:384]
            nc.vector.tensor_scalar_add(t2, s0, 1.0)
            nc.vector.tensor_mul(t2, t2, t1)
            t3 = sq[:, 384:512]
            nc.vector.reciprocal(t3, t2)
            nc.vector.tensor_mul(t3, t3, s0)
            nc.vector.tensor_tensor(
                u_b[:].rearrange("p (i d) -> p i d", d=D),
                x_sb[:].rearrange("p (i d) -> p i d", d=D),
                t3.rearrange("p (i one) -> p i one", one=1
                             ).broadcast_to((128, 128, D)), op=OP.mult)

            # uT via 16 PE transposes of [128,128] chunks
            uT_v = uT[:].rearrange("p (iq m2 b) -> p m2 b iq", iq=4, m2=16)
            for m in range(16):
                tps = psp.tile([128, 512], f32, tag="ps")
                nc.tensor.transpose(tps[:, 0:128],
                                    u_b[:, 128 * m:128 * (m + 1)], ident[:])
                nc.vector.tensor_copy(
                    uT_v[:, m],
                    tps[:, 0:128].rearrange("p (b iq) -> p b iq", iq=4))

        # BD: block-diagonal u tiles (zeros persist off-diagonal).
        # 16-row partition bands aren't engine-addressable (32-align rule),
        # so scatter with SBUF->SBUF DMAs on rotating queues.
        BD_v = BD[:].rearrange("p (g2 i4 b) -> p g2 i4 b", i4=4, b=B)
        uT_g = uT[:].rearrange("p (g2 b) -> p g2 b", b=B)
        qs = [nc.sync, nc.scalar]
        for gpar in range(2):
            for i4 in range(4):
                rows = slice(gpar * 64 + i4 * 16, gpar * 64 + i4 * 16 + 16)
                qs[(gpar * 4 + i4) % 2].dma_start(BD_v[rows, :, i4],
                                                  uT_g[rows])

        # stream W (bf16, pre-transposed on host); build u_hat and s0
        u_hat_v = u_hat[:].rearrange("p (h c j l) -> p h c j l",
                                     h=IGH, c=C, j=J, l=IGL)
        s0ps = s0p.tile([32, CJ], f32, tag="s0")
        with tc.tile_pool(name="wstream", bufs=2) as wsp:
            CH = 8
            for t in range(G2 // CH):
                wst = wsp.tile([128, CH * CJ], bf16, tag="wst")
                (nc.sync if t % 2 == 0 else nc.scalar).dma_start(
                    wst[:], w_in[:, t * CH * CJ:(t + 1) * CH * CJ])
                for q in range(CH):
                    g2 = t * CH + q
                    rhs_full = wst[:, q * CJ:(q + 1) * CJ]
                    for gpar in range(2):
                        g = 2 * g2 + gpar
                        ps = evp.tile([128, 512], f32, tag="ev")
                        nc.tensor.matmul(
                            ps[:, 0:CJ],
                            BD[gpar * 64:(gpar + 1) * 64,
                               g2 * 128:(g2 + 1) * 128],
                            wst[gpar * 64:(gpar + 1) * 64,
                                q * CJ:(q + 1) * CJ],
                            start=True, stop=True)
                        dst = u_hat_v[:, g // IGL, :, :, g % IGL]
                        src = ps[:, 0:CJ].rearrange("p (c j) -> p c j", j=J)
                        if gpar == 0:
                            nc.vector.tensor_copy(dst, src)
                        else:
                            nc.scalar.copy(dst, src)
                    # s0 accumulation: K=128 spans both gpar halves
                    nc.tensor.matmul(s0ps[:, 0:CJ],
                                     uT[:, g2 * B:(g2 + 1) * B], rhs_full,
                                     start=(g2 == 0), stop=(g2 == G2 - 1))

    # ---------- routing ----------
    prodp = ctx.enter_context(tc.tile_pool(name="prod", bufs=2))
    prodp3 = ctx.enter_context(tc.tile_pool(name="prod3", bufs=3))
    rscr = ctx.enter_context(tc.tile_pool(name="rscr", bufs=1))
    eb = rscr.tile([128, IGH * C * IGL], bf16)          # 8KB/p
    v_exp = rscr.tile([128, C * J * IGL], bf16)         # 4KB/p
    NCH = 16
    CC = C // NCH  # 2

    u_hat_v = u_hat[:].rearrange("p (h c j l) -> p h c j l",
                                 h=IGH, c=C, j=J, l=IGL)
    cw_b = cw[:].rearrange("p (h c one l) -> p h c one l",
                           h=IGH, c=C, one=1, l=IGL
                           ).broadcast_to((128, IGH, C, J, IGL))
    bl_v = bl[:].rearrange("p (h c l) -> p h c l", h=IGH, c=C)
    eb_v = eb[:].rearrange("p (h c l) -> p h c l", h=IGH, c=C)
    cw_v = cw[:].rearrange("p (h c l) -> p h c l", h=IGH, c=C)
    ve_b = v_exp[:].rearrange("p (one c j l) -> p one c j l",
                              one=1, c=C, j=J
                              ).broadcast_to((128, IGH, C, J, IGL))

    HC = C // 2  # capsules per AR half
    HCJ = HC * J

    def half_send(k, half):
        """i4 partition-reduce for one c-half, then issue its AllReduce."""
        cols = slice(half * HCJ, (half + 1) * HCJ)
        if k == 0:
            sps, prow = s0ps, s0ps[0:32, cols]
        else:
            sps = psp.tile([128, 512], f32, tag="ps")
            nc.tensor.matmul(sps[0:32, 0:HCJ], msk[:], s_pre[:, cols],
                             start=True, stop=True)
            prow = sps[0:32, 0:HCJ]
        scale = (1.0 / C) if k == 0 else 1.0
        nc.scalar.activation(s_sb[:, cols], prow, ACTF.Copy,
                             bias=0.0, scale=scale)
        ar_i = dram.tile([32, HCJ], f32, tag=f"ari{k}h{half}")
        ar_o = dram.tile([32, HCJ], f32, tag=f"aro{k}h{half}")
        nc.sync.dma_start(ar_i[:], s_sb[:, cols])
        if no_cc:
            nc.sync.dma_start(ar_o[:], ar_i[:])
        else:
            nc.gpsimd.collective_compute(
                "AllReduce", OP.add, replica_groups=REPL,
                ins=[ar_i.opt()], outs=[ar_o.opt()])
        return ar_o

    def half_recv(ar_o, half):
        """Receive AR result, squash that c-half -> v_sb columns."""
        cols = slice(half * HCJ, (half + 1) * HCJ)
        ccols = slice(half * HC, (half + 1) * HC)
        nc.sync.dma_start(s_sb[:, cols], ar_o[:])
        nrm = sqs[:, 0:C][:, ccols]
        nc.vector.tensor_mul(v_sq[:, cols], s_sb[:, cols], s_sb[:, cols])
        nc.vector.tensor_reduce(
            nrm, v_sq[:, cols].rearrange("p (c j) -> p c j", j=J),
            axis=AX.X, op=OP.add)
        t1 = sqs[:, C:2 * C][:, ccols]
        nc.scalar.activation(t1, nrm, ACTF.Sqrt, bias=cst[0:32, 0:1],
                             scale=1.0)
        t2 = sqs[:, 2 * C:3 * C][:, ccols]
        nc.vector.tensor_scalar_add(t2, nrm, 1.0)
        nc.vector.tensor_mul(t2, t2, t1)
        t3 = sqs[:, 3 * C:4 * C][:, ccols]
        nc.vector.reciprocal(t3, t2)
        nc.vector.tensor_mul(t3, t3, nrm)
        nc.vector.tensor_tensor(
            v_sb[:, cols].rearrange("p (c j) -> p c j", j=J),
            s_sb[:, cols].rearrange("p (c j) -> p c j", j=J),
            t3.rearrange("p (c one) -> p c one", one=1
                         ).broadcast_to((32, HC, J)), op=OP.mult)

    def b_prep_half(half):
        # v-half -> bf16 -> replicate over i4 partitions -> expand over igL
        cols = slice(half * HCJ, (half + 1) * HCJ)
        nc.scalar.copy(vbf[:, cols], v_sb[:, cols])
        qs = [nc.sync, nc.scalar, nc.gpsimd]
        for i4 in range(4):
            qs[i4 % 3].dma_start(v_rep[32 * i4:32 * (i4 + 1), cols],
                                 vbf[:, cols])
        ve_v = v_exp[:].rearrange("p (c j l) -> p c j l", c=C, j=J)[
            :, half * HC:(half + 1) * HC]
        nc.gpsimd.tensor_copy(
            ve_v[:, :, :, 0:1],
            v_rep[:, cols].rearrange("p (c j one) -> p c j one",
                                     j=J, one=1))
        w = 1
        while w < IGL:
            nc.gpsimd.tensor_copy(ve_v[:, :, :, w:2 * w], ve_v[:, :, :, 0:w])
            w *= 2


    def b_pass_half(half):
        for th in range(NCH // 2):
            t = half * (NCH // 2) + th
            cs = slice(t * CC, (t + 1) * CC)
            prod = prodp.tile([128, IGH * CC * J * IGL], bf16, tag="pr2")
            pv = prod[:].rearrange("p (h c j l) -> p h c j l",
                                   h=IGH, c=CC, j=J)
            eng = (nc.gpsimd if (half == 0 and th in (2, 6))
                   or (half == 1 and th == 4) else nc.vector)
            eng.tensor_tensor(pv, u_hat_v[:, :, cs], ve_b[:, :, cs],
                              op=OP.mult)
            w = J // 2
            while w >= 1:
                eng.tensor_tensor(pv[:, :, :, 0:w], pv[:, :, :, 0:w],
                                  pv[:, :, :, w:2 * w], op=OP.add)
                w //= 2
            nc.gpsimd.tensor_tensor(bl_v[:, :, cs], bl_v[:, :, cs],
                                     pv[:, :, :, 0], op=OP.add)

    def softmax_exp_half(half):
        cs = slice(half * HC, (half + 1) * HC)
        nc.scalar.activation(eb_v[:, :, cs], bl_v[:, :, cs], ACTF.Exp,
                             bias=cst[:, 1:2], scale=1.0)

    def softmax():
        # cw = eb / sum_c eb (eb halves computed right after each b-half)
        nc.vector.tensor_reduce(
            zsum[:].rearrange("p (h l) -> p h l", h=IGH),
            eb[:].rearrange("p (h c l) -> p h l c", h=IGH, c=C),
            axis=AX.X, op=OP.add)
        nc.vector.reciprocal(zi[:], zsum[:])
        nc.scalar.copy(zibf[:], zi[:])
        nc.vector.tensor_tensor(
            cw_v, eb_v,
            zibf[:].rearrange("p (h one l) -> p h one l", h=IGH, one=1
                              ).broadcast_to((128, IGH, C, IGL)),
            op=OP.mult)

    def s_mult_half(k, half):
        for c in range(half * HC, (half + 1) * HC):
            prod = prodp3.tile([128, IGH * J * IGL], bf16, tag="pr")
            # j-major product: memory (j, h, l) so (h,l)=128 is contiguous
            pv = prod[:].rearrange("p (j h l) -> p h j l", h=IGH, j=J)
            eng = nc.gpsimd if c % 5 == 2 else nc.vector
            eng.tensor_tensor(pv, u_hat_v[:, :, c], cw_b[:, :, c],
                              op=OP.mult)
            with nc.allow_low_precision("bf16 s_pre (internal fp32 accum)"):
                nc.vector.tensor_reduce(
                    s_pre[:, c * J:(c + 1) * J],
                    prod[:].rearrange("p (j hl) -> p j hl", j=J),
                    axis=AX.X, op=OP.add)

    for k in range(ROUTINGS):
        if k > 0:
            ars = []
            for half in (0, 1):
                s_mult_half(k, half)
                ars.append(half_send(k, half))
        else:
            ars = [half_send(0, 0), half_send(0, 1)]
        for half in (0, 1):
            half_recv(ars[half], half)
            if k < ROUTINGS - 1:
                b_prep_half(half)
                b_pass_half(half)
                softmax_exp_half(half)
        if k < ROUTINGS - 1:
            softmax()

    nc.sync.dma_start(v_out.rearrange("b c j -> b (c j)"), v_sb[:])
    ctx.close()


# ======================= runner =======================
import types
import concourse.bacc as bacc
from concourse import bass_utils


def _install_ntff_hook():
    """The agent image lacks antenv.axon_hooks; build it from the boot
    shim's ctypes NTFF driver so trace=True yields real HW profiles."""
    if "antenv.axon_hooks" in sys.modules:
        return
    try:
        sys.path.insert(0, "/root/.axon_site")
        from trn_agent_boot.trn_boot import _ntff_profile_via_ctypes
        hook = _ntff_profile_via_ctypes("/opt/axon/libaxon_pjrt.so")
        if hook is None:
            return
        m = types.ModuleType("antenv.axon_hooks")
        m.get_axon_ntff_profile_hook = lambda: hook
        m.set_axon_ntff_profile_hook = lambda h: None
        sys.modules["antenv.axon_hooks"] = m
    except Exception:
        pass


_CACHE = {}


def _build():
    if "nc" in _CACHE:
        return _CACHE["nc"]
    nc = bacc.Bacc("TRN2", target_bir_lowering=False, debug=False,
                   enable_asserts=False, num_devices=N_CORES)
    x_d = nc.dram_tensor("x", (128, NI * D // 4), f32,
                         kind="ExternalInput").ap()
    w_d = nc.dram_tensor("W", (128, G2 * CJ), bf16,
                         kind="ExternalInput").ap()
    v_d = nc.dram_tensor("v", (B, C, J), f32, kind="ExternalOutput").ap()
    with tile.TileContext(nc) as tc:
        build_capsule_kernel(tc, v_d, x_d, w_d)
    nc.compile()
    _CACHE["nc"] = nc
    return nc


def kernel(x: np.ndarray, W: np.ndarray) -> np.ndarray:
    import ml_dtypes
    x = np.ascontiguousarray(x, dtype=np.float32)
    W = np.ascontiguousarray(W, dtype=np.float32)
    nc = _build()
    in_maps = []
    for k in range(N_CORES):
        xs = np.ascontiguousarray(
            x[:, k * NI * D:(k + 1) * NI * D]).reshape(128, NI * D // 4)
        ws = W[:, k * NI:(k + 1) * NI]  # [C, NI, J, D]
        wt = np.ascontiguousarray(
            ws.reshape(C, G2, 2, 4, J, D).transpose(2, 3, 5, 1, 0, 4)
            .reshape(128, G2 * CJ)).astype(ml_dtypes.bfloat16)
        in_maps.append({"x": xs, "W": wt})
    do_trace = os.environ.get("CAPS_TRACE", "0") == "1"
    if do_trace:
        _install_ntff_hook()
    res = bass_utils.run_bass_kernel_spmd(
        nc, in_maps, core_ids=list(range(N_CORES)), trace=do_trace,
        tmpdir=os.environ.get("CAPS_TRACE_DIR") or None)
    if res.exec_time_ns is not None:
        print(f"HW exec time: {res.exec_time_ns} ns")
    return res.results[0]["v"]


# revision 17
# speedup vs baseline: 1.0344x; 1.0344x over previous
import sys
sys.path.insert(0, "/opt/trn_rl_repo")
"""CapsuleBlock kernel for TRN2, i-sharded across 8 cores (v3: resident u_hat).

Per-core (NI=512 input capsules local):
  u = squash(x); u_hat[b,c,i,j] computed ONCE via block-diagonal-u matmuls
  (K=(i'4,d)=64, M=(i4,b)=128, N=(c,j)=512) and stored bf16 in SBUF as
  u_hat[(i4,b), (igH, c, j, igL)] with local i = 4*(igH*IGL+igL) + i4.
  s0 (uniform coupling) is accumulated on PE during the same W stream.
  Routing runs on DVE/GpSimd:
    s-pass: prod = cw (.) u_hat (j-major prod), one X-reduce over (igH,igL),
            i4-partition-reduce via mask matmul, 64KB AllReduce, squash.
    b-pass: prod2 = v (.) u_hat, j-tree-add, bl += ., softmax over c -> cw.
  A dummy AllReduce at kernel start absorbs the cold-collective cost.

Host-side prep (layout only): x fed as [(b,iq)=128, (ir,d)=2048] f32;
W fed pre-transposed/cast as [(gpar,i4,d)=128, (g2,c,j)=32768] bf16.
"""

import os
import numpy as np
from contextlib import ExitStack

import concourse.bass as bass
import concourse.mybir as mybir
import concourse.tile as tile
from concourse import masks

f32 = mybir.dt.float32
bf16 = mybir.dt.bfloat16
AX = mybir.AxisListType
OP = mybir.AluOpType
ACTF = mybir.ActivationFunctionType

B, C, J, D = 32, 32, 16, 16
ROUTINGS = 3
EPS = 1e-7
N_CORES = 8
NI = 4096 // N_CORES          # 512 per core
G = NI // 4                   # 128 groups of 4 i's
G2 = G // 2                   # 64
IGH, IGL = 32, 4              # g = igH*IGL + igL
CJ = C * J                    # 512
REPL = [list(range(N_CORES))]


def build_capsule_kernel(tc: tile.TileContext, v_out: bass.AP, x_in: bass.AP,
                         w_in: bass.AP):
    """v_out [B, C, J] f32; x_in [128, 2048] f32; w_in [128, G2*CJ] bf16."""
    nc = tc.nc
    no_cc = os.environ.get("NO_COLLECTIVE") == "1"

    ctx = ExitStack()
    main = ctx.enter_context(tc.tile_pool(name="main", bufs=1))
    psp = ctx.enter_context(tc.tile_pool(name="ps", bufs=1, space="PSUM"))
    evp = ctx.enter_context(tc.tile_pool(name="evp", bufs=3, space="PSUM"))
    s0p = ctx.enter_context(tc.tile_pool(name="s0p", bufs=1, space="PSUM"))
    dram = ctx.enter_context(tc.tile_pool(name="dram", bufs=1, space="DRAM"))

    # ---------- persistent SBUF ----------
    u_hat = main.tile([128, IGH * C * J * IGL], bf16)   # 128KB/p
    bl = main.tile([128, IGH * C * IGL], f32)           # 16KB/p
    cw = main.tile([128, IGH * C * IGL], bf16)          # 8KB/p
    s_pre = main.tile([128, CJ], bf16)                  # 1KB/p
    zsum = main.tile([128, IGH * IGL], f32)
    zi = main.tile([128, IGH * IGL], f32)
    zibf = main.tile([128, IGH * IGL], bf16)
    v_rep = main.tile([128, CJ], bf16)
    s_sb = main.tile([32, CJ], f32)
    v_sb = main.tile([32, CJ], f32)
    v_sq = main.tile([32, CJ], f32)
    vbf = main.tile([32, CJ], bf16)
    sqs = main.tile([32, 4 * C], f32)
    msk = main.tile([128, 32], bf16)
    cst = main.tile([128, 2], f32)

    nc.vector.memset(cst[:, 0:1], EPS)
    nc.vector.memset(cst[:, 1:2], 0.0)
    nc.vector.memset(bl[:], 0.0)

    # warm up the collective ring early; latency hides under init
    if not no_cc:
        war_i = dram.tile([32, 2], f32, tag="wari")
        war_o = dram.tile([32, 2], f32, tag="waro")
        nc.sync.dma_start(war_i[:], cst[0:32, 0:2])
        nc.gpsimd.collective_compute(
            "AllReduce", OP.add, replica_groups=REPL,
            ins=[war_i.opt()], outs=[war_o.opt()])

    # ---------- init: x -> u -> uT -> BD; stream W -> u_hat, s0 ----------
    with tc.tile_pool(name="init2", bufs=1) as initp2:
        ident = initp2.tile([128, 128], f32)
        ident_bf = initp2.tile([32, 32], bf16)
        uT = initp2.tile([128, G2 * B], bf16)    # [(gpar,i4,d), (g2,b)]
        BD = initp2.tile([128, G2 * 128], bf16)  # [(gpar,i4,d), (g2,i4',b)]
        masks.make_identity(nc, ident[:])
        masks.make_identity(nc, ident_bf[:])
        for k4 in range(4):
            nc.sync.dma_start(msk[32 * k4:32 * (k4 + 1), :], ident_bf[:])
        nc.vector.memset(BD[:], 0.0)

        with tc.tile_pool(name="init1", bufs=1) as initp1:
            x_sb = initp1.tile([128, 2048], f32)     # [(b,iq), (ir,d)]
            u_b = initp1.tile([128, 2048], f32)
            sq = initp1.tile([128, 512], f32)
            nc.sync.dma_start(x_sb[:], x_in[:])

            # squash over d for each of 128 local-i per partition
            nc.vector.tensor_mul(u_b[:], x_sb[:], x_sb[:])
            s0 = sq[:, 0:128]
            nc.vector.tensor_reduce(s0,
                                    u_b[:].rearrange("p (i d) -> p i d", d=D),
                                    axis=AX.X, op=OP.add)
            t1 = sq[:, 128:256]
            nc.scalar.activation(t1, s0, ACTF.Sqrt, bias=cst[:, 0:1],
                                 scale=1.0)
            t2 = sq[:, 256:384]
            nc.vector.tensor_scalar_add(t2, s0, 1.0)
            nc.vector.tensor_mul(t2, t2, t1)
            t3 = sq[:, 384:512]
            nc.vector.reciprocal(t3, t2)
            nc.vector.tensor_mul(t3, t3, s0)
            nc.vector.tensor_tensor(
                u_b[:].rearrange("p (i d) -> p i d", d=D),
                x_sb[:].rearrange("p (i d) -> p i d", d=D),
                t3.rearrange("p (i one) -> p i one", one=1
                             ).broadcast_to((128, 128, D)), op=OP.mult)

            # uT via 16 PE transposes of [128,128] chunks
            uT_v = uT[:].rearrange("p (iq m2 b) -> p m2 b iq", iq=4, m2=16)
            for m in range(16):
                tps = psp.tile([128, 512], f32, tag="ps")
                nc.tensor.transpose(tps[:, 0:128],
                                    u_b[:, 128 * m:128 * (m + 1)], ident[:])
                nc.vector.tensor_copy(
                    uT_v[:, m],
                    tps[:, 0:128].rearrange("p (b iq) -> p b iq", iq=4))

        # BD: block-diagonal u tiles (zeros persist off-diagonal).
        # 16-row partition bands aren't engine-addressable (32-align rule),
        # so scatter with SBUF->SBUF DMAs on rotating queues.
        BD_v = BD[:].rearrange("p (g2 i4 b) -> p g2 i4 b", i4=4, b=B)
        uT_g = uT[:].rearrange("p (g2 b) -> p g2 b", b=B)
        qs = [nc.sync, nc.scalar]
        for gpar in range(2):
            for i4 in range(4):
                rows = slice(gpar * 64 + i4 * 16, gpar * 64 + i4 * 16 + 16)
                qs[(gpar * 4 + i4) % 2].dma_start(BD_v[rows, :, i4],
                                                  uT_g[rows])

        # stream W (bf16, pre-transposed on host); build u_hat and s0
        u_hat_v = u_hat[:].rearrange("p (h c j l) -> p h c j l",
                                     h=IGH, c=C, j=J, l=IGL)
        s0ps = s0p.tile([32, CJ], f32, tag="s0")
        with tc.tile_pool(name="wstream", bufs=2) as wsp:
            CH = 8
            for t in range(G2 // CH):
                wst = wsp.tile([128, CH * CJ], bf16, tag="wst")
                (nc.sync if t % 2 == 0 else nc.scalar).dma_start(
                    wst[:], w_in[:, t * CH * CJ:(t + 1) * CH * CJ])
                for q in range(CH):
                    g2 = t * CH + q
                    rhs_full = wst[:, q * CJ:(q + 1) * CJ]
                    ps = evp.tile([128, 1024], f32, tag="ev")
                    for gpar in range(2):
                        nc.tensor.matmul(
                            ps[:, gpar * CJ:(gpar + 1) * CJ],
                            BD[gpar * 64:(gpar + 1) * 64,
                               g2 * 128:(g2 + 1) * 128],
                            wst[gpar * 64:(gpar + 1) * 64,
                                q * CJ:(q + 1) * CJ],
                            start=True, stop=True)
                    # evacuate the g-pair (same igH, igL pair 2*g2%IGL..+1):
                    # psum-side strided reads, SBUF writes in 4B runs
                    g = 2 * g2
                    dst = u_hat_v[:, g // IGL, :, :,
                                  (g % IGL):(g % IGL) + 2]
                    srcv = ps[:, 0:1024].rearrange(
                        "p (gb c j) -> p c j gb", gb=2, j=J)
                    if g2 % 2 == 0:
                        nc.vector.tensor_copy(dst, srcv)
                    else:
                        nc.scalar.copy(dst, srcv)
                    # s0 accumulation: K=128 spans both gpar halves
                    nc.tensor.matmul(s0ps[:, 0:CJ],
                                     uT[:, g2 * B:(g2 + 1) * B], rhs_full,
                                     start=(g2 == 0), stop=(g2 == G2 - 1))

    # ---------- routing ----------
    prodp = ctx.enter_context(tc.tile_pool(name="prod", bufs=2))
    prodp3 = ctx.enter_context(tc.tile_pool(name="prod3", bufs=3))
    rscr = ctx.enter_context(tc.tile_pool(name="rscr", bufs=1))
    eb = rscr.tile([128, IGH * C * IGL], bf16)          # 8KB/p
    v_exp = rscr.tile([128, C * J * IGL], bf16)         # 4KB/p
    NCH = 16
    CC = C // NCH  # 2

    u_hat_v = u_hat[:].rearrange("p (h c j l) -> p h c j l",
                                 h=IGH, c=C, j=J, l=IGL)
    cw_b = cw[:].rearrange("p (h c one l) -> p h c one l",
                           h=IGH, c=C, one=1, l=IGL
                           ).broadcast_to((128, IGH, C, J, IGL))
    bl_v = bl[:].rearrange("p (h c l) -> p h c l", h=IGH, c=C)
    eb_v = eb[:].rearrange("p (h c l) -> p h c l", h=IGH, c=C)
    cw_v = cw[:].rearrange("p (h c l) -> p h c l", h=IGH, c=C)
    ve_b = v_exp[:].rearrange("p (one c j l) -> p one c j l",
                              one=1, c=C, j=J
                              ).broadcast_to((128, IGH, C, J, IGL))

    HC = C // 2  # capsules per AR half
    HCJ = HC * J

    def half_send(k, half):
        """i4 partition-reduce for one c-half, then issue its AllReduce."""
        cols = slice(half * HCJ, (half + 1) * HCJ)
        if k == 0:
            sps, prow = s0ps, s0ps[0:32, cols]
        else:
            sps = psp.tile([128, 512], f32, tag="ps")
            nc.tensor.matmul(sps[0:32, 0:HCJ], msk[:], s_pre[:, cols],
                             start=True, stop=True)
            prow = sps[0:32, 0:HCJ]
        scale = (1.0 / C) if k == 0 else 1.0
        nc.scalar.activation(s_sb[:, cols], prow, ACTF.Copy,
                             bias=0.0, scale=scale)
        ar_i = dram.tile([32, HCJ], f32, tag=f"ari{k}h{half}")
        ar_o = dram.tile([32, HCJ], f32, tag=f"aro{k}h{half}")
        nc.sync.dma_start(ar_i[:], s_sb[:, cols])
        if no_cc:
            nc.sync.dma_start(ar_o[:], ar_i[:])
        else:
            nc.gpsimd.collective_compute(
                "AllReduce", OP.add, replica_groups=REPL,
                ins=[ar_i.opt()], outs=[ar_o.opt()])
        return ar_o

    def half_recv(ar_o, half):
        """Receive AR result, squash that c-half -> v_sb columns."""
        cols = slice(half * HCJ, (half + 1) * HCJ)
        ccols = slice(half * HC, (half + 1) * HC)
        nc.sync.dma_start(s_sb[:, cols], ar_o[:])
        nrm = sqs[:, 0:C][:, ccols]
        nc.vector.tensor_mul(v_sq[:, cols], s_sb[:, cols], s_sb[:, cols])
        nc.vector.tensor_reduce(
            nrm, v_sq[:, cols].rearrange("p (c j) -> p c j", j=J),
            axis=AX.X, op=OP.add)
        t1 = sqs[:, C:2 * C][:, ccols]
        nc.scalar.activation(t1, nrm, ACTF.Sqrt, bias=cst[0:32, 0:1],
                             scale=1.0)
        t2 = sqs[:, 2 * C:3 * C][:, ccols]
        nc.vector.tensor_scalar_add(t2, nrm, 1.0)
        nc.vector.tensor_mul(t2, t2, t1)
        t3 = sqs[:, 3 * C:4 * C][:, ccols]
        nc.vector.reciprocal(t3, t2)
        nc.vector.tensor_mul(t3, t3, nrm)
        nc.vector.tensor_tensor(
            v_sb[:, cols].rearrange("p (c j) -> p c j", j=J),
            s_sb[:, cols].rearrange("p (c j) -> p c j", j=J),
            t3.rearrange("p (c one) -> p c one", one=1
                         ).broadcast_to((32, HC, J)), op=OP.mult)

    def b_prep_half(half):
        # v-half -> bf16 -> replicate over i4 partitions -> expand over igL
        cols = slice(half * HCJ, (half + 1) * HCJ)
        nc.scalar.copy(vbf[:, cols], v_sb[:, cols])
        qs = [nc.sync, nc.scalar, nc.gpsimd]
        for i4 in range(4):
            qs[i4 % 3].dma_start(v_rep[32 * i4:32 * (i4 + 1), cols],
                                 vbf[:, cols])
        ve_v = v_exp[:].rearrange("p (c j l) -> p c j l", c=C, j=J)[
            :, half * HC:(half + 1) * HC]
        nc.gpsimd.tensor_copy(
            ve_v[:, :, :, 0:1],
            v_rep[:, cols].rearrange("p (c j one) -> p c j one",
                                     j=J, one=1))
        w = 1
        while w < IGL:
            nc.gpsimd.tensor_copy(ve_v[:, :, :, w:2 * w], ve_v[:, :, :, 0:w])
            w *= 2


    def b_pass_half(half):
        for th in range(NCH // 2):
            t = half * (NCH // 2) + th
            cs = slice(t * CC, (t + 1) * CC)
            prod = prodp.tile([128, IGH * CC * J * IGL], bf16, tag="pr2")
            pv = prod[:].rearrange("p (h c j l) -> p h c j l",
                                   h=IGH, c=CC, j=J)
            eng = (nc.gpsimd if (half == 0 and th in (2, 6))
                   or (half == 1 and th == 4) else nc.vector)
            eng.tensor_tensor(pv, u_hat_v[:, :, cs], ve_b[:, :, cs],
                              op=OP.mult)
            w = J // 2
            while w >= 1:
                eng.tensor_tensor(pv[:, :, :, 0:w], pv[:, :, :, 0:w],
                                  pv[:, :, :, w:2 * w], op=OP.add)
                w //= 2
            nc.gpsimd.tensor_tensor(bl_v[:, :, cs], bl_v[:, :, cs],
                                     pv[:, :, :, 0], op=OP.add)

    def softmax_exp_half(half):
        cs = slice(half * HC, (half + 1) * HC)
        nc.scalar.activation(eb_v[:, :, cs], bl_v[:, :, cs], ACTF.Exp,
                             bias=cst[:, 1:2], scale=1.0)

    def softmax():
        # cw = eb / sum_c eb (eb halves computed right after each b-half)
        nc.vector.tensor_reduce(
            zsum[:].rearrange("p (h l) -> p h l", h=IGH),
            eb[:].rearrange("p (h c l) -> p h l c", h=IGH, c=C),
            axis=AX.X, op=OP.add)
        nc.vector.reciprocal(zi[:], zsum[:])
        nc.scalar.copy(zibf[:], zi[:])
        nc.vector.tensor_tensor(
            cw_v, eb_v,
            zibf[:].rearrange("p (h one l) -> p h one l", h=IGH, one=1
                              ).broadcast_to((128, IGH, C, IGL)),
            op=OP.mult)

    def s_mult_half(k, half):
        for c in range(half * HC, (half + 1) * HC):
            prod = prodp3.tile([128, IGH * J * IGL], bf16, tag="pr")
            # j-major product: memory (j, h, l) so (h,l)=128 is contiguous
            pv = prod[:].rearrange("p (j h l) -> p h j l", h=IGH, j=J)
            eng = nc.gpsimd if c % 5 == 2 else nc.vector
            eng.tensor_tensor(pv, u_hat_v[:, :, c], cw_b[:, :, c],
                              op=OP.mult)
            with nc.allow_low_precision("bf16 s_pre (internal fp32 accum)"):
                nc.vector.tensor_reduce(
                    s_pre[:, c * J:(c + 1) * J],
                    prod[:].rearrange("p (j hl) -> p j hl", j=J),
                    axis=AX.X, op=OP.add)

    for k in range(ROUTINGS):
        if k > 0:
            ars = []
            for half in (0, 1):
                s_mult_half(k, half)
                ars.append(half_send(k, half))
        else:
            ars = [half_send(0, 0), half_send(0, 1)]
        for half in (0, 1):
            half_recv(ars[half], half)
            if k < ROUTINGS - 1:
                b_prep_half(half)
                b_pass_half(half)
                softmax_exp_half(half)
        if k < ROUTINGS - 1:
            softmax()

    nc.sync.dma_start(v_out.rearrange("b c j -> b (c j)"), v_sb[:])
    ctx.close()


# ======================= runner =======================
import types
import concourse.bacc as bacc
from concourse import bass_utils


def _install_ntff_hook():
    """The agent image lacks antenv.axon_hooks; build it from the boot
    shim's ctypes NTFF driver so trace=True yields real HW profiles."""
    if "antenv.axon_hooks" in sys.modules:
        return
    try:
        sys.path.insert(0, "/root/.axon_site")
        from trn_agent_boot.trn_boot import _ntff_profile_via_ctypes
        hook = _ntff_profile_via_ctypes("/opt/axon/libaxon_pjrt.so")
        if hook is None:
            return
        m = types.ModuleType("antenv.axon_hooks")
        m.get_axon_ntff_profile_hook = lambda: hook
        m.set_axon_ntff_profile_hook = lambda h: None
        sys.modules["antenv.axon_hooks"] = m
    except Exception:
        pass


_CACHE = {}


def _build():
    if "nc" in _CACHE:
        return _CACHE["nc"]
    nc = bacc.Bacc("TRN2", target_bir_lowering=False, debug=False,
                   enable_asserts=False, num_devices=N_CORES)
    x_d = nc.dram_tensor("x", (128, NI * D // 4), f32,
                         kind="ExternalInput").ap()
    w_d = nc.dram_tensor("W", (128, G2 * CJ), bf16,
                         kind="ExternalInput").ap()
    v_d = nc.dram_tensor("v", (B, C, J), f32, kind="ExternalOutput").ap()
    with tile.TileContext(nc) as tc:
        build_capsule_kernel(tc, v_d, x_d, w_d)
    nc.compile()
    _CACHE["nc"] = nc
    return nc


def kernel(x: np.ndarray, W: np.ndarray) -> np.ndarray:
    import ml_dtypes
    x = np.ascontiguousarray(x, dtype=np.float32)
    W = np.ascontiguousarray(W, dtype=np.float32)
    nc = _build()
    in_maps = []
    for k in range(N_CORES):
        xs = np.ascontiguousarray(
            x[:, k * NI * D:(k + 1) * NI * D]).reshape(128, NI * D // 4)
        ws = W[:, k * NI:(k + 1) * NI]  # [C, NI, J, D]
        wt = np.ascontiguousarray(
            ws.reshape(C, G2, 2, 4, J, D).transpose(2, 3, 5, 1, 0, 4)
            .reshape(128, G2 * CJ)).astype(ml_dtypes.bfloat16)
        in_maps.append({"x": xs, "W": wt})
    do_trace = os.environ.get("CAPS_TRACE", "0") == "1"
    if do_trace:
        _install_ntff_hook()
    res = bass_utils.run_bass_kernel_spmd(
        nc, in_maps, core_ids=list(range(N_CORES)), trace=do_trace,
        tmpdir=os.environ.get("CAPS_TRACE_DIR") or None)
    if res.exec_time_ns is not None:
        print(f"HW exec time: {res.exec_time_ns} ns")
    return res.results[0]["v"]
